# revision 35
# baseline (speedup 1.0000x reference)
"""Multi-head self-attention (pre-LN) Trainium2 kernel, 8-way sharded.

Sharding: batch (2) x head-groups (4 groups of 4 heads) = 8 shards, one per
NeuronCore. Each core computes LayerNorm on its batch slice, column-sharded
Q/K/V projections (256 cols = 4 heads x 64), flash-style attention for its 4
heads, and a row-sharded output projection producing a partial [2048, 1024]
output. The host sums the 4 head-group partials per batch and adds bo.

Matmul operands are bf16 (full PE rate + fast weight load); accumulation is
always fp32 in PSUM. The softmax denominator reciprocal/broadcast runs in
float32r to keep the per-row normalization accurate.
"""

import sys

for _p in ("/opt/trn_rl_repo",):
    if _p not in sys.path:
        sys.path.append(_p)

import numpy as np

import concourse.bass as bass
import concourse.mybir as mybir
import concourse.tile as tile
from concourse import bacc
from concourse.masks import make_identity

F32 = mybir.dt.float32
F32R = mybir.dt.float32r
BF16 = mybir.dt.bfloat16

S = 2048          # sequence length per batch
D = 1024          # model dim
COLS = 256        # cols per core (4 heads x 64)
HEADS = 4         # heads per core
HDIM = 64
NSB = S // 512    # 4 seq blocks of 512
NST = S // 128    # 16 seq tiles of 128
NDT = D // 128    # 8 d tiles of 128
SCALE = 1.0 / np.sqrt(64.0)


def build_nc():
    nc = bacc.Bacc("TRN2", target_bir_lowering=False, debug=False)

    x_d = nc.declare_dram_parameter("x", [S, D], F32, isOutput=False)
    wq_d = nc.declare_dram_parameter("wq", [D, COLS], F32, isOutput=False)
    wk_d = nc.declare_dram_parameter("wk", [D, COLS], F32, isOutput=False)
    wv_d = nc.declare_dram_parameter("wv", [D, COLS], F32, isOutput=False)
    wo_d = nc.declare_dram_parameter("wo", [COLS, D], F32, isOutput=False)
    bq_d = nc.declare_dram_parameter("bq", [1, COLS], F32, isOutput=False)
    bk_d = nc.declare_dram_parameter("bk", [1, COLS], F32, isOutput=False)
    bv_d = nc.declare_dram_parameter("bv", [1, COLS], F32, isOutput=False)
    gam_d = nc.declare_dram_parameter("gamma", [D], F32, isOutput=False)
    bet_d = nc.declare_dram_parameter("beta", [D], F32, isOutput=False)
    out_d = nc.declare_dram_parameter("out", [S, D], F32, isOutput=True)

    Alu = mybir.AluOpType
    Act = mybir.ActivationFunctionType

    with (
        nc.allow_low_precision(reason="bf16 matmul operands by design"),
        tile.TileContext(nc) as tc,
    ):
        with (
            tc.tile_pool(name="persist", bufs=1) as persist,
            tc.tile_pool(name="prep", bufs=1) as prep,
            tc.tile_pool(name="x_pool", bufs=3) as x_pool,
            tc.tile_pool(name="z_pool", bufs=6) as z_pool,
            tc.tile_pool(name="zt_pool", bufs=3) as zt_pool,
            tc.tile_pool(name="smallA", bufs=8) as smallA,
            tc.tile_pool(name="exp_pool", bufs=8) as exp_pool,
            tc.tile_pool(name="smallB", bufs=4) as smallB,
            tc.tile_pool(name="out_pool", bufs=3) as out_pool,
        ):
            ident_b = persist.tile([128, 128], BF16, tag="ident_b")
            make_identity(nc, ident_b)
            ones_b = persist.tile([1, 512], BF16, tag="ones_b")
            eps_sb = persist.tile([128, 1], F32, tag="eps")
            nc.vector.memset(eps_sb, 1e-5)
            w_sbs = {
                nm: persist.tile([128, NDT, COLS], BF16, tag=f"w{nm}", name=f"w{nm}")
                for nm in ("q", "k", "v")
            }
            wo_sb = persist.tile([128, 2, D], BF16, tag="wo")
            bps = {
                nm: persist.tile([1, COLS], BF16, tag=f"bp{nm}", name=f"bp{nm}")
                for nm in "qkv"
            }
            qT_sb = persist.tile([128, 2, S], BF16, tag="qT")
            kT_sb = persist.tile([128, 2, S], BF16, tag="kT")
            oT_sb = persist.tile([128, 2, S], BF16, tag="oT")
            # V natural [kseq, head, 64 + ones column]
            v_sb = persist.tile([128, NST, HEADS, HDIM + 1], BF16, tag="v")

            # ---------------- Prep: load/cast weights, fold gamma/beta ----
            with tc.tile_pool(name="ps_bias", bufs=1, space="PSUM") as ps_bias:
                ones_f32 = prep.tile([1, 512], F32, tag="ones_f32")
                nc.vector.memset(ones_f32, 1.0)
                nc.vector.tensor_copy(ones_b, ones_f32)
                vones_f32 = prep.tile([128, NST, HEADS, 1], F32, tag="vones")
                nc.vector.memset(vones_f32, 1.0)
                nc.vector.tensor_copy(v_sb[:, :, :, HDIM : HDIM + 1], vones_f32)

                gam_sb = prep.tile([128, NDT], F32, tag="gam")
                nc.scalar.dma_start(gam_sb, gam_d.rearrange("(o p) -> p o", p=128))
                bet_raw = prep.tile([128, NDT], F32, tag="bet_raw")
                nc.scalar.dma_start(bet_raw, bet_d.rearrange("(o p) -> p o", p=128))
                bet_sb = prep.tile([128, NDT], BF16, tag="bet")
                nc.vector.tensor_copy(bet_sb, bet_raw)

                # weights, gamma-folded + cast: W'[d, c] = gamma[d] * W[d, c]
                # (on ScalarE: activation Copy with per-partition scale, so the
                # VectorE is free for LayerNorm from t=0)
                for nm, wd in (("q", wq_d), ("k", wk_d), ("v", wv_d)):
                    w_raw = prep.tile([128, NDT, COLS], F32, tag=f"wraw{nm}")
                    nc.scalar.dma_start(w_raw, wd.rearrange("(o p) c -> p o c", p=128))
                    w_sb = w_sbs[nm]
                    for dt in range(NDT):
                        nc.scalar.activation(
                            w_sb[:, dt, :],
                            w_raw[:, dt, :],
                            Act.Copy,
                            scale=gam_sb[:, dt : dt + 1],
                        )
                wo_raw = prep.tile([128, 2, D], F32, tag="wo_raw")
                nc.scalar.dma_start(wo_raw, wo_d.rearrange("(t p) n -> p t n", p=128))
                nc.scalar.copy(wo_sb, wo_raw)

                # effective biases: b'[c] = beta @ W' + b
                for nm, bd in (("q", bq_d), ("k", bk_d), ("v", bv_d)):
                    braw = prep.tile([1, COLS], F32, tag=f"braw{nm}")
                    nc.scalar.dma_start(braw, bd[:, :])
                    bp_ps = ps_bias.tile([1, COLS], F32, tag="bias_ps")
                    w_sb = w_sbs[nm]
                    for dt in range(NDT):
                        nc.tensor.matmul(
                            bp_ps,
                            lhsT=bet_sb[:, dt : dt + 1],
                            rhs=w_sb[:, dt, :],
                            start=(dt == 0),
                            stop=(dt == NDT - 1),
                        )
                    nc.vector.tensor_tensor(bps[nm], bp_ps, braw, Alu.add)

            # ---------------- Phase A: LN -> transpose -> Q/K/V ----------
            with (
                tc.tile_pool(name="ps_t", bufs=2, space="PSUM") as ps_t,
                tc.tile_pool(name="ps_mm", bufs=1, space="PSUM") as ps_mm,
            ):
                for sb in range(NSB):
                    zT_blk = zt_pool.tile([128, NDT, 512], BF16, tag="zT")
                    z_ts = []
                    for j in range(4):
                        st = sb * 4 + j
                        x_t = x_pool.tile([128, D], F32, tag="x")
                        nc.sync.dma_start(x_t, x_d[st * 128 : (st + 1) * 128, :])
                        stats = smallA.tile([128, 2, 6], F32, tag="stats")
                        nc.vector.bn_stats(stats[:, 0, :], x_t[:, :512])
                        nc.vector.bn_stats(stats[:, 1, :], x_t[:, 512:])
                        mv = smallA.tile([128, 2], F32, tag="mv")
                        nc.vector.bn_aggr(mv, stats)
                        rstd = smallA.tile([128, 1], F32, tag="rstd")
                        nc.scalar.activation(rstd, mv[:, 1:2], Act.Sqrt, bias=eps_sb)
                        nc.vector.reciprocal(rstd, rstd)
                        z_t = z_pool.tile([128, D], BF16, tag="z")
                        nc.vector.tensor_scalar(
                            z_t,
                            x_t,
                            scalar1=mv[:, 0:1],
                            scalar2=rstd,
                            op0=Alu.subtract,
                            op1=Alu.mult,
                        )
                        z_ts.append(z_t)
                    # Interleave per d-tile: 4 transposes, then the Q/K/V
                    # matmuls consuming that d-tile (keeps the PE stream dense
                    # so HAM stays warm). Grouped psum accumulators:
                    #   qacc/kacc [128, 2, 512] (2 banks each), vacc
                    #   [128, 4, 256] (2 banks).
                    qacc = ps_mm.tile([128, 2, 512], F32, tag="qacc")
                    kacc = ps_mm.tile([128, 2, 512], F32, tag="kacc")
                    accs = {"q": qacc, "k": kacc}
                    for dt in range(NDT):
                        tp = ps_t.tile([128, 512], BF16, tag="tp")
                        for j in range(4):
                            nc.tensor.transpose(
                                tp[:, j * 128 : (j + 1) * 128],
                                z_ts[j][:, dt * 128 : (dt + 1) * 128],
                                ident_b,
                            )
                        nc.scalar.copy(zT_blk[:, dt, :], tp)
                        for nm in ("q", "k"):
                            for cp in range(2):
                                nc.tensor.matmul(
                                    accs[nm][:, cp, :],
                                    lhsT=w_sbs[nm][:, dt, cp * 128 : (cp + 1) * 128],
                                    rhs=zT_blk[:, dt, :],
                                    start=(dt == 0),
                                    stop=False,
                                )
                    # biases (rank-1 matmuls close each accumulation group)
                    for nm in ("q", "k"):
                        for cp in range(2):
                            nc.tensor.matmul(
                                accs[nm][:, cp, :],
                                lhsT=bps[nm][:, cp * 128 : (cp + 1) * 128],
                                rhs=ones_b,
                                start=False,
                                stop=True,
                            )
                    nc.scalar.copy(qT_sb[:, :, sb * 512 : (sb + 1) * 512], qacc)
                    nc.scalar.copy(kT_sb[:, :, sb * 512 : (sb + 1) * 512], kacc)
                    # V rows for this seq block (dense PE clump right after
                    # the QK stream; zT_blk is fully materialized by now)
                    for j in range(4):
                        st = sb * 4 + j
                        ps = ps_t.tile([128, COLS], F32, tag="vps")
                        for dt in range(NDT):
                            nc.tensor.matmul(
                                ps,
                                lhsT=zT_blk[:, dt, j * 128 : (j + 1) * 128],
                                rhs=w_sbs["v"][:, dt, :],
                                start=(dt == 0),
                                stop=False,
                            )
                        nc.tensor.matmul(
                            ps,
                            lhsT=ones_b[:, :128],
                            rhs=bps["v"],
                            start=False,
                            stop=True,
                        )
                        nc.scalar.copy(
                            v_sb[:, st, :, :HDIM],
                            ps.rearrange("p (h e) -> p h e", h=HEADS),
                        )

            # ---------------- Phase B: attention + output projection -----
            # kst pairs: two back-to-back score matmuls into a 2-bank psum
            # tile, one wide exp, two AV accumulate matmuls.
            with (
                tc.tile_pool(name="ps_sc", bufs=3, space="PSUM") as ps_sc,
                tc.tile_pool(name="ps_ot", bufs=1, space="PSUM") as ps_ot,
                tc.tile_pool(name="ps_out", bufs=1, space="PSUM") as ps_out,
            ):
                for qb in range(NSB):
                    for h in range(HEADS):
                        hp = 64 * (h % 2)
                        cp = h // 2
                        qslc = qT_sb[hp : hp + 64, cp, qb * 512 : (qb + 1) * 512]
                        otp = ps_ot.tile([HDIM + 1, 512], F32, tag="ot")
                        for kg in range(NST // 2):
                            scp = ps_sc.tile([128, 2, 512], F32, tag="sc")
                            for u in range(2):
                                kst = 2 * kg + u
                                nc.tensor.matmul(
                                    scp[:, u, :],
                                    lhsT=kT_sb[
                                        hp : hp + 64, cp, kst * 128 : (kst + 1) * 128
                                    ],
                                    rhs=qslc,
                                    start=True,
                                    stop=True,
                                )
                            et = exp_pool.tile([128, 2, 512], BF16, tag="et")
                            nc.scalar.activation(et, scp, Act.Exp, scale=SCALE)
                            for u in range(2):
                                kst = 2 * kg + u
                                nc.tensor.matmul(
                                    otp,
                                    lhsT=v_sb[:, kst, h, :],
                                    rhs=et[:, u, :],
                                    start=(kst == 0),
                                    stop=(kst == NST - 1),
                                )
                        # evict the accumulator right away to free the PSUM
                        # bank; normalization then runs off the critical path
                        # entirely on the (otherwise idle) GPSIMD engine
                        ot_sbuf = smallB.tile([HDIM + 1, 512], F32, tag="ot_sbuf")
                        nc.vector.tensor_copy(ot_sbuf, otp)
                        recip = smallB.tile([1, 512], F32, tag="recip")
                        nc.vector.reciprocal(recip, ot_sbuf[HDIM : HDIM + 1, :])
                        bc = smallB.tile([64, 512], F32, tag="bc")
                        nc.gpsimd.partition_broadcast(bc, recip)
                        nc.vector.tensor_tensor(
                            oT_sb[hp : hp + 64, cp, qb * 512 : (qb + 1) * 512],
                            ot_sbuf[:HDIM, :],
                            bc,
                            Alu.mult,
                        )
                    # output projection for this query block (overlaps the
                    # next block's attention)
                    for st in range(4 * qb, 4 * (qb + 1)):
                        for nck in range(2):
                            ps = ps_out.tile([128, 512], F32, tag="op")
                            for cp in range(2):
                                nc.tensor.matmul(
                                    ps,
                                    lhsT=oT_sb[:, cp, st * 128 : (st + 1) * 128],
                                    rhs=wo_sb[:, cp, nck * 512 : (nck + 1) * 512],
                                    start=(cp == 0),
                                    stop=(cp == 1),
                                )
                            ot = out_pool.tile([128, 512], F32, tag="out")
                            if nck == 0:
                                nc.scalar.copy(ot, ps)
                            else:
                                nc.vector.tensor_copy(ot, ps)
                            nc.sync.dma_start(
                                out_d[
                                    st * 128 : (st + 1) * 128,
                                    nck * 512 : (nck + 1) * 512,
                                ],
                                ot,
                            )
    nc.compile()
    return nc


_NC_CACHE = None


def _get_nc():
    global _NC_CACHE
    if _NC_CACHE is None:
        _NC_CACHE = build_nc()
    return _NC_CACHE


def shard_inputs(inputs):
    x = np.ascontiguousarray(np.asarray(inputs["x"], dtype=np.float32))
    in_maps = []
    for core in range(8):
        b, hg = core // 4, core % 4
        cols = slice(hg * COLS, (hg + 1) * COLS)
        in_maps.append(
            {
                "x": x[b],
                "wq": np.ascontiguousarray(inputs["Wq"][:, cols], dtype=np.float32),
                "wk": np.ascontiguousarray(inputs["Wk"][:, cols], dtype=np.float32),
                "wv": np.ascontiguousarray(inputs["Wv"][:, cols], dtype=np.float32),
                "wo": np.ascontiguousarray(inputs["Wo"][cols, :], dtype=np.float32),
                "bq": np.asarray(inputs["bq"][cols], dtype=np.float32).reshape(1, COLS),
                "bk": np.asarray(inputs["bk"][cols], dtype=np.float32).reshape(1, COLS),
                "bv": np.asarray(inputs["bv"][cols], dtype=np.float32).reshape(1, COLS),
                "gamma": np.asarray(inputs["ln_gamma"], dtype=np.float32),
                "beta": np.asarray(inputs["ln_beta"], dtype=np.float32),
            }
        )
    return in_maps


def run(inputs, trace=False):
    from concourse.bass_utils import run_bass_kernel_spmd

    nc = _get_nc()
    in_maps = shard_inputs(inputs)
    res = run_bass_kernel_spmd(nc, in_maps, core_ids=list(range(8)), trace=trace)
    parts = np.stack([res.results[i]["out"] for i in range(8)])  # [8, S, D]
    out = parts.reshape(2, 4, S, D).sum(axis=1)
    out = out + np.asarray(inputs["bo"], dtype=np.float32)[None, None, :]
    return out.astype(np.float32), res


def kernel(**inputs):
    return run(inputs)[0]


# revision 36
# speedup vs baseline: 1.0858x; 1.0858x over previous
"""Multi-head self-attention (pre-LN) Trainium2 kernel, 8-way sharded.

Sharding: batch (2) x head-groups (4 groups of 4 heads) = 8 shards, one per
NeuronCore. Each core computes LayerNorm on its batch slice, column-sharded
Q/K/V projections (256 cols = 4 heads x 64), flash-style attention for its 4
heads, and a row-sharded output projection producing a partial [2048, 1024]
output. The host sums the 4 head-group partials per batch and adds bo.

Matmul operands are bf16 (full PE rate + fast weight load); accumulation is
always fp32 in PSUM. The softmax denominator reciprocal/broadcast runs in
float32r to keep the per-row normalization accurate.
"""

import sys

for _p in ("/opt/trn_rl_repo",):
    if _p not in sys.path:
        sys.path.append(_p)

import numpy as np

import concourse.bass as bass
import concourse.mybir as mybir
import concourse.tile as tile
from concourse import bacc
from concourse.masks import make_identity

F32 = mybir.dt.float32
F32R = mybir.dt.float32r
BF16 = mybir.dt.bfloat16

S = 2048          # sequence length per batch
D = 1024          # model dim
COLS = 256        # cols per core (4 heads x 64)
HEADS = 4         # heads per core
HDIM = 64
NSB = S // 512    # 4 seq blocks of 512
NST = S // 128    # 16 seq tiles of 128
NDT = D // 128    # 8 d tiles of 128
SCALE = 1.0 / np.sqrt(64.0)


def build_nc():
    nc = bacc.Bacc("TRN2", target_bir_lowering=False, debug=False)

    x_d = nc.declare_dram_parameter("x", [S, D], F32, isOutput=False)
    wq_d = nc.declare_dram_parameter("wq", [D, COLS], F32, isOutput=False)
    wk_d = nc.declare_dram_parameter("wk", [D, COLS], F32, isOutput=False)
    wv_d = nc.declare_dram_parameter("wv", [D, COLS], F32, isOutput=False)
    wo_d = nc.declare_dram_parameter("wo", [COLS, D], F32, isOutput=False)
    bq_d = nc.declare_dram_parameter("bq", [1, COLS], F32, isOutput=False)
    bk_d = nc.declare_dram_parameter("bk", [1, COLS], F32, isOutput=False)
    bv_d = nc.declare_dram_parameter("bv", [1, COLS], F32, isOutput=False)
    gam_d = nc.declare_dram_parameter("gamma", [D], F32, isOutput=False)
    bet_d = nc.declare_dram_parameter("beta", [D], F32, isOutput=False)
    out_d = nc.declare_dram_parameter("out", [S, D], F32, isOutput=True)

    Alu = mybir.AluOpType
    Act = mybir.ActivationFunctionType

    with (
        nc.allow_low_precision(reason="bf16 matmul operands by design"),
        tile.TileContext(nc) as tc,
    ):
        with (
            tc.tile_pool(name="persist", bufs=1) as persist,
            tc.tile_pool(name="prep", bufs=1) as prep,
            tc.tile_pool(name="x_pool", bufs=3) as x_pool,
            tc.tile_pool(name="z_pool", bufs=6) as z_pool,
            tc.tile_pool(name="zt_pool", bufs=3) as zt_pool,
            tc.tile_pool(name="smallA", bufs=8) as smallA,
            tc.tile_pool(name="exp_pool", bufs=8) as exp_pool,
            tc.tile_pool(name="smallB", bufs=4) as smallB,
            tc.tile_pool(name="out_pool", bufs=3) as out_pool,
        ):
            ident_b = persist.tile([128, 128], BF16, tag="ident_b")
            make_identity(nc, ident_b)
            ones_b = persist.tile([1, 512], BF16, tag="ones_b")
            eps_sb = persist.tile([128, 1], F32, tag="eps")
            nc.vector.memset(eps_sb, 1e-5)
            w_sbs = {
                nm: persist.tile([128, NDT, COLS], BF16, tag=f"w{nm}", name=f"w{nm}")
                for nm in ("q", "k", "v")
            }
            wo_sb = persist.tile([128, 2, D], BF16, tag="wo")
            bps = {
                nm: persist.tile([1, COLS], BF16, tag=f"bp{nm}", name=f"bp{nm}")
                for nm in "qkv"
            }
            qT_sb = persist.tile([128, 2, S], BF16, tag="qT")
            kT_sb = persist.tile([128, 2, S], BF16, tag="kT")
            oT_sb = persist.tile([128, 2, S], BF16, tag="oT")
            # V natural [kseq, head, 64 + ones column]
            v_sb = persist.tile([128, NST, HEADS, HDIM + 1], BF16, tag="v")

            # ---------------- Prep: load/cast weights, fold gamma/beta ----
            with tc.tile_pool(name="ps_bias", bufs=1, space="PSUM") as ps_bias:
                ones_f32 = prep.tile([1, 512], F32, tag="ones_f32")
                nc.vector.memset(ones_f32, 1.0)
                nc.vector.tensor_copy(ones_b, ones_f32)
                vones_f32 = prep.tile([128, NST, HEADS, 1], F32, tag="vones")
                nc.vector.memset(vones_f32, 1.0)
                nc.vector.tensor_copy(v_sb[:, :, :, HDIM : HDIM + 1], vones_f32)

                gam_sb = prep.tile([128, NDT], F32, tag="gam")
                nc.scalar.dma_start(gam_sb, gam_d.rearrange("(o p) -> p o", p=128))
                bet_raw = prep.tile([128, NDT], F32, tag="bet_raw")
                nc.scalar.dma_start(bet_raw, bet_d.rearrange("(o p) -> p o", p=128))
                bet_sb = prep.tile([128, NDT], BF16, tag="bet")
                nc.vector.tensor_copy(bet_sb, bet_raw)

                # weights, gamma-folded + cast: W'[d, c] = gamma[d] * W[d, c]
                # (on ScalarE: activation Copy with per-partition scale, so the
                # VectorE is free for LayerNorm from t=0)
                for nm, wd in (("q", wq_d), ("k", wk_d), ("v", wv_d)):
                    w_raw = prep.tile([128, NDT, COLS], F32, tag=f"wraw{nm}")
                    nc.scalar.dma_start(w_raw, wd.rearrange("(o p) c -> p o c", p=128))
                    w_sb = w_sbs[nm]
                    for dt in range(NDT):
                        nc.scalar.activation(
                            w_sb[:, dt, :],
                            w_raw[:, dt, :],
                            Act.Copy,
                            scale=gam_sb[:, dt : dt + 1],
                        )
                wo_raw = prep.tile([128, 2, D], F32, tag="wo_raw")
                nc.scalar.dma_start(wo_raw, wo_d.rearrange("(t p) n -> p t n", p=128))
                nc.scalar.copy(wo_sb, wo_raw)

                # effective biases: b'[c] = beta @ W' + b
                for nm, bd in (("q", bq_d), ("k", bk_d), ("v", bv_d)):
                    braw = prep.tile([1, COLS], F32, tag=f"braw{nm}")
                    nc.scalar.dma_start(braw, bd[:, :])
                    bp_ps = ps_bias.tile([1, COLS], F32, tag="bias_ps")
                    w_sb = w_sbs[nm]
                    for dt in range(NDT):
                        nc.tensor.matmul(
                            bp_ps,
                            lhsT=bet_sb[:, dt : dt + 1],
                            rhs=w_sb[:, dt, :],
                            start=(dt == 0),
                            stop=(dt == NDT - 1),
                        )
                    nc.vector.tensor_tensor(bps[nm], bp_ps, braw, Alu.add)

            # ---------------- Phase A: LN -> transpose -> Q/K/V ----------
            with (
                tc.tile_pool(name="ps_t", bufs=2, space="PSUM") as ps_t,
                tc.tile_pool(name="ps_mm", bufs=1, space="PSUM") as ps_mm,
            ):
                for sb in range(NSB):
                    zT_blk = zt_pool.tile([128, NDT, 512], BF16, tag="zT")
                    z_ts = []
                    for j in range(4):
                        st = sb * 4 + j
                        x_t = x_pool.tile([128, D], F32, tag="x")
                        nc.sync.dma_start(x_t, x_d[st * 128 : (st + 1) * 128, :])
                        stats = smallA.tile([128, 2, 6], F32, tag="stats")
                        nc.vector.bn_stats(stats[:, 0, :], x_t[:, :512])
                        nc.vector.bn_stats(stats[:, 1, :], x_t[:, 512:])
                        mv = smallA.tile([128, 2], F32, tag="mv")
                        nc.vector.bn_aggr(mv, stats)
                        rstd = smallA.tile([128, 1], F32, tag="rstd")
                        nc.scalar.activation(rstd, mv[:, 1:2], Act.Sqrt, bias=eps_sb)
                        nc.vector.reciprocal(rstd, rstd)
                        z_t = z_pool.tile([128, D], BF16, tag="z")
                        nc.vector.tensor_scalar(
                            z_t,
                            x_t,
                            scalar1=mv[:, 0:1],
                            scalar2=rstd,
                            op0=Alu.subtract,
                            op1=Alu.mult,
                        )
                        z_ts.append(z_t)
                    # Interleave per d-tile: 4 transposes, then the Q/K/V
                    # matmuls consuming that d-tile (keeps the PE stream dense
                    # so HAM stays warm). Grouped psum accumulators:
                    #   qacc/kacc [128, 2, 512] (2 banks each), vacc
                    #   [128, 4, 256] (2 banks).
                    qacc = ps_mm.tile([128, 2, 512], F32, tag="qacc")
                    kacc = ps_mm.tile([128, 2, 512], F32, tag="kacc")
                    accs = {"q": qacc, "k": kacc}
                    for dt in range(NDT):
                        tp = ps_t.tile([128, 512], BF16, tag="tp")
                        for j in range(4):
                            nc.tensor.transpose(
                                tp[:, j * 128 : (j + 1) * 128],
                                z_ts[j][:, dt * 128 : (dt + 1) * 128],
                                ident_b,
                            )
                        nc.scalar.copy(zT_blk[:, dt, :], tp)
                        for nm in ("q", "k"):
                            for cp in range(2):
                                nc.tensor.matmul(
                                    accs[nm][:, cp, :],
                                    lhsT=w_sbs[nm][:, dt, cp * 128 : (cp + 1) * 128],
                                    rhs=zT_blk[:, dt, :],
                                    start=(dt == 0),
                                    stop=False,
                                )
                    # biases (rank-1 matmuls close each accumulation group)
                    for nm in ("q", "k"):
                        for cp in range(2):
                            nc.tensor.matmul(
                                accs[nm][:, cp, :],
                                lhsT=bps[nm][:, cp * 128 : (cp + 1) * 128],
                                rhs=ones_b,
                                start=False,
                                stop=True,
                            )
                    nc.scalar.copy(qT_sb[:, :, sb * 512 : (sb + 1) * 512], qacc)
                    nc.scalar.copy(kT_sb[:, :, sb * 512 : (sb + 1) * 512], kacc)
                    # V rows for this seq block (dense PE clump right after
                    # the QK stream; zT_blk is fully materialized by now)
                    for j in range(4):
                        st = sb * 4 + j
                        ps = ps_t.tile([128, COLS], F32, tag="vps")
                        for dt in range(NDT):
                            nc.tensor.matmul(
                                ps,
                                lhsT=zT_blk[:, dt, j * 128 : (j + 1) * 128],
                                rhs=w_sbs["v"][:, dt, :],
                                start=(dt == 0),
                                stop=False,
                            )
                        nc.tensor.matmul(
                            ps,
                            lhsT=ones_b[:, :128],
                            rhs=bps["v"],
                            start=False,
                            stop=True,
                        )
                        nc.scalar.copy(
                            v_sb[:, st, :, :HDIM],
                            ps.rearrange("p (h e) -> p h e", h=HEADS),
                        )

            # ---------------- Phase B: attention + output projection -----
            # kst pairs: two back-to-back score matmuls into a 2-bank psum
            # tile, one wide exp, two AV accumulate matmuls.
            with (
                tc.tile_pool(name="ps_sc", bufs=2, space="PSUM") as ps_sc,
                tc.tile_pool(name="ps_ot", bufs=2, space="PSUM") as ps_ot,
                tc.tile_pool(name="ps_out", bufs=2, space="PSUM") as ps_out,
            ):
                for qb in range(NSB):
                    for h in range(HEADS):
                        hp = 64 * (h % 2)
                        cp = h // 2
                        qslc = qT_sb[hp : hp + 64, cp, qb * 512 : (qb + 1) * 512]
                        otp = ps_ot.tile([HDIM + 1, 512], F32, tag="ot")
                        for kg in range(NST // 2):
                            scp = ps_sc.tile([128, 2, 512], F32, tag="sc")
                            for u in range(2):
                                kst = 2 * kg + u
                                nc.tensor.matmul(
                                    scp[:, u, :],
                                    lhsT=kT_sb[
                                        hp : hp + 64, cp, kst * 128 : (kst + 1) * 128
                                    ],
                                    rhs=qslc,
                                    start=True,
                                    stop=True,
                                )
                            et = exp_pool.tile([128, 2, 512], BF16, tag="et")
                            nc.scalar.activation(et, scp, Act.Exp, scale=SCALE)
                            for u in range(2):
                                kst = 2 * kg + u
                                nc.tensor.matmul(
                                    otp,
                                    lhsT=v_sb[:, kst, h, :],
                                    rhs=et[:, u, :],
                                    start=(kst == 0),
                                    stop=(kst == NST - 1),
                                )
                        # evict the accumulator right away to free the PSUM
                        # bank; normalization then runs off the critical path
                        # entirely on the (otherwise idle) GPSIMD engine
                        ot_sbuf = smallB.tile([HDIM + 1, 512], F32, tag="ot_sbuf")
                        nc.vector.tensor_copy(ot_sbuf, otp)
                        recip = smallB.tile([1, 512], F32, tag="recip")
                        nc.vector.reciprocal(recip, ot_sbuf[HDIM : HDIM + 1, :])
                        bc = smallB.tile([64, 512], F32, tag="bc")
                        nc.gpsimd.partition_broadcast(bc, recip)
                        nc.vector.tensor_tensor(
                            oT_sb[hp : hp + 64, cp, qb * 512 : (qb + 1) * 512],
                            ot_sbuf[:HDIM, :],
                            bc,
                            Alu.mult,
                        )
                    # output projection for this query block (overlaps the
                    # next block's attention)
                    for st in range(4 * qb, 4 * (qb + 1)):
                        for nck in range(2):
                            ps = ps_out.tile([128, 512], F32, tag="op")
                            for cp in range(2):
                                nc.tensor.matmul(
                                    ps,
                                    lhsT=oT_sb[:, cp, st * 128 : (st + 1) * 128],
                                    rhs=wo_sb[:, cp, nck * 512 : (nck + 1) * 512],
                                    start=(cp == 0),
                                    stop=(cp == 1),
                                )
                            ot = out_pool.tile([128, 512], F32, tag="out")
                            if nck == 0:
                                nc.scalar.copy(ot, ps)
                            else:
                                nc.vector.tensor_copy(ot, ps)
                            nc.sync.dma_start(
                                out_d[
                                    st * 128 : (st + 1) * 128,
                                    nck * 512 : (nck + 1) * 512,
                                ],
                                ot,
                            )
    nc.compile()
    return nc


_NC_CACHE = None


def _get_nc():
    global _NC_CACHE
    if _NC_CACHE is None:
        _NC_CACHE = build_nc()
    return _NC_CACHE


def shard_inputs(inputs):
    x = np.ascontiguousarray(np.asarray(inputs["x"], dtype=np.float32))
    in_maps = []
    for core in range(8):
        b, hg = core // 4, core % 4
        cols = slice(hg * COLS, (hg + 1) * COLS)
        in_maps.append(
            {
                "x": x[b],
                "wq": np.ascontiguousarray(inputs["Wq"][:, cols], dtype=np.float32),
                "wk": np.ascontiguousarray(inputs["Wk"][:, cols], dtype=np.float32),
                "wv": np.ascontiguousarray(inputs["Wv"][:, cols], dtype=np.float32),
                "wo": np.ascontiguousarray(inputs["Wo"][cols, :], dtype=np.float32),
                "bq": np.asarray(inputs["bq"][cols], dtype=np.float32).reshape(1, COLS),
                "bk": np.asarray(inputs["bk"][cols], dtype=np.float32).reshape(1, COLS),
                "bv": np.asarray(inputs["bv"][cols], dtype=np.float32).reshape(1, COLS),
                "gamma": np.asarray(inputs["ln_gamma"], dtype=np.float32),
                "beta": np.asarray(inputs["ln_beta"], dtype=np.float32),
            }
        )
    return in_maps


def run(inputs, trace=False):
    from concourse.bass_utils import run_bass_kernel_spmd

    nc = _get_nc()
    in_maps = shard_inputs(inputs)
    res = run_bass_kernel_spmd(nc, in_maps, core_ids=list(range(8)), trace=trace)
    parts = np.stack([res.results[i]["out"] for i in range(8)])  # [8, S, D]
    out = parts.reshape(2, 4, S, D).sum(axis=1)
    out = out + np.asarray(inputs["bo"], dtype=np.float32)[None, None, :]
    return out.astype(np.float32), res


def kernel(**inputs):
    return run(inputs)[0]
